# revision 12
# baseline (speedup 1.0000x reference)
"""MoE domain-gate routing kernel for Trainium2 (8 NeuronCores, token-sharded).

Computes, for T=8192 tokens, E=16 experts, capacity C=512:
  combine[t, e, c] = 1.0 iff token t routed to expert e at capacity slot c
  dispatch = combine != 0
  l_aux = 0.0

Per-core plan (1024 tokens each):
  * Host merges (domain_ids, mask) into a single f32 "mids" stream
    (padded tokens -> sentinel 100), laid out [128 partitions, 64 blocks]
    with the 64 token-blocks ROTATED so each core's own 8 blocks come
    first; a per-core 0/1 prefix matrix W encodes which (rotated) blocks
    precede each local block globally. This keeps the device program
    identical across cores (SPMD) with all per-core variation in data.
  * Device: block-expert counts from a transposed one-hot (blocks on
    partitions) reduced along the free dim; exclusive per-(local block,
    expert) offsets via one matmul with W; within-block inclusive cumsum
    via matmul with upper-triangular ones; offsets broadcast along
    partitions with K=1 matmuls. From the resulting global positions:
    capacity filter, slot index, flat index idx = e*512 + slot (or a
    sentinel if dropped).
  * Output: for each 128-token row block, compare an iota row [0..8191]
    (DMA-loaded constant) against idx per partition -> [128, 8192] f32
    combine tile (+ u8 cast on the scalar engine for dispatch), DMA to
    DRAM on both HWDGE rings. 40MB of output per core; memory-bound.
"""

import numpy as np

import concourse.bacc as bacc
import concourse.mybir as mybir
import concourse.tile as tile
from concourse.bass_utils import run_bass_kernel_spmd

_T, _E, _C = 8192, 16, 512
_NC = 8  # cores
_B = 64  # global 128-token blocks
_BL = 8  # local blocks per core
_TL = _B // _NC * 128  # tokens per core = 1024
_W = _E * _C  # flat output row width = 8192
_SENT = 1.0e6  # flat-index sentinel for dropped/padded tokens

_f32 = mybir.dt.float32
_u8 = mybir.dt.uint8

_nc_cache = []
_iota_cache = []


def _build_nc():
    nc = bacc.Bacc("TRN2", target_bir_lowering=False, debug=False, num_devices=_NC)

    mids_d = nc.dram_tensor("mids", [128, _B], _f32, kind="ExternalInput")
    midst_d = nc.dram_tensor("midst", [_B, 128], _f32, kind="ExternalInput")
    w_d = nc.dram_tensor("w", [_B, _BL], _f32, kind="ExternalInput")
    u_d = nc.dram_tensor("u", [128, 128], _f32, kind="ExternalInput")
    sel8_d = nc.dram_tensor("sel8", [_BL, _BL * 128], _f32, kind="ExternalInput")
    iota_d = nc.dram_tensor("iota", [128, _W], _f32, kind="ExternalInput")
    comb_d = nc.dram_tensor("comb", [_TL, _W], _f32, kind="ExternalOutput")
    disp_d = nc.dram_tensor("disp", [_TL, _W], _u8, kind="ExternalOutput")

    with tile.TileContext(nc) as tc:
        with (
            tc.tile_pool(name="consts", bufs=1) as cpool,
            tc.tile_pool(name="work", bufs=1) as wpool,
            tc.tile_pool(name="bigc", bufs=3) as bigc,
            tc.tile_pool(name="bigd", bufs=3) as bigd,
            tc.tile_pool(name="psum", bufs=1, space="PSUM") as ppool,
        ):
            iota_t = cpool.tile([128, _W], _f32)
            nc.sync.dma_start(iota_t[:], iota_d[:])
            mids_t = cpool.tile([128, _B], _f32)
            midst_t = cpool.tile([_B, 128], _f32)
            w_t = cpool.tile([_B, _BL], _f32)
            u_t = cpool.tile([128, 128], _f32)
            sel8_t = cpool.tile([_BL, _BL * 128], _f32)
            nc.sync.dma_start(mids_t[:], mids_d[:])
            nc.sync.dma_start(midst_t[:], midst_d[:])
            nc.sync.dma_start(w_t[:], w_d[:])
            nc.sync.dma_start(u_t[:], u_d[:])
            nc.sync.dma_start(sel8_t[:], sel8_d[:])

            # transposed one-hot X2[r, e*128+p] = (midsT[r, p] == e) -> block
            # counts bsums[r, e] by reducing over p (innermost)
            x2_t = wpool.tile([_B, _E * 128], _f32)
            x23 = x2_t[:].rearrange("r (e p) -> r e p", p=128)
            for e in range(_E):
                nc.vector.tensor_single_scalar(
                    x23[:, e, :], midst_t[:], float(e), mybir.AluOpType.is_equal
                )
            bsums_t = wpool.tile([_B, _E], _f32)
            nc.vector.reduce_sum(bsums_t[:], x23[:, :, :], axis=mybir.AxisListType.X)

            # local one-hot X[p, j*16+e] = (mids[p, j] == e), j = local block
            x_t = wpool.tile([128, _BL * _E], _f32)
            x3 = x_t[:].rearrange("p (j e) -> p j e", e=_E)
            for e in range(_E):
                nc.vector.tensor_single_scalar(
                    x3[:, :, e], mids_t[:, 0:_BL], float(e), mybir.AluOpType.is_equal
                )

            # exclusive offsets per (local block, expert): W^T @ bsums -> [8, 16]
            o_p = ppool.tile([_BL, _E], _f32)
            nc.tensor.matmul(o_p[:], w_t[:], bsums_t[:], start=True, stop=True)
            o_s = wpool.tile([_BL, _E], _f32)
            nc.vector.tensor_copy(o_s[:], o_p[:])

            # p1 = within-block inclusive cumsum; p2 = offsets broadcast along
            # partitions (K=1 matmuls per local block)
            # p2 = offsets broadcast along partitions; copied to SBUF so the
            # final add reads only one PSUM operand
            p2 = ppool.tile([128, 128], _f32)
            for j in range(_BL):
                nc.tensor.matmul(
                    p2[:, j * _E : (j + 1) * _E],
                    sel8_t[:, j * 128 : (j + 1) * 128],
                    o_s[:],
                    start=True,
                    stop=True,
                )
            o2_t = wpool.tile([128, 128], _f32)
            nc.vector.tensor_copy(o2_t[:], p2[:])
            p1 = ppool.tile([128, 128], _f32)
            nc.tensor.matmul(p1[:], u_t[:], x_t[:], start=True, stop=True)
            pos_t = wpool.tile([128, 128], _f32)
            nc.vector.tensor_add(pos_t[:], p1[:], o2_t[:])

            # capacity filter and slot index
            sel = wpool.tile([128, 128], _f32)
            nc.vector.tensor_single_scalar(
                sel[:], pos_t[:], float(_C), mybir.AluOpType.is_le
            )
            g_t = wpool.tile([128, 128], _f32)
            nc.vector.tensor_mul(g_t[:], sel[:], x_t[:])
            pos1 = wpool.tile([128, 128], _f32)
            nc.vector.tensor_scalar_add(pos1[:], pos_t[:], -1.0)
            locg = wpool.tile([128, 128], _f32)
            nc.vector.tensor_mul(locg[:], pos1[:], g_t[:])
            loc_s = wpool.tile([128, _BL], _f32)
            nc.vector.reduce_sum(
                loc_s[:],
                locg[:].rearrange("p (j e) -> p j e", e=_E),
                axis=mybir.AxisListType.X,
            )
            kept = wpool.tile([128, _BL], _f32)
            nc.vector.reduce_sum(
                kept[:],
                g_t[:].rearrange("p (j e) -> p j e", e=_E),
                axis=mybir.AxisListType.X,
            )
            # idx = (e*512 + slot) if kept else SENT
            t1 = wpool.tile([128, _BL], _f32)
            nc.vector.tensor_single_scalar(
                t1[:], mids_t[:, 0:_BL], float(_C), mybir.AluOpType.mult
            )
            t2 = wpool.tile([128, _BL], _f32)
            nc.vector.tensor_add(t2[:], t1[:], loc_s[:])
            t3 = wpool.tile([128, _BL], _f32)
            nc.vector.tensor_mul(t3[:], t2[:], kept[:])
            t4 = wpool.tile([128, _BL], _f32)
            nc.vector.tensor_scalar(
                t4[:], kept[:], -_SENT, _SENT, mybir.AluOpType.mult, mybir.AluOpType.add
            )
            idx_t = wpool.tile([128, _BL], _f32)
            nc.vector.tensor_add(idx_t[:], t3[:], t4[:])

            # output generation: one 128-token row block at a time; comb DMAs
            # on the sync HWDGE ring, disp DMAs on the scalar HWDGE ring,
            # both chopped so no single tail transfer dominates one queue.
            for j in range(_BL):
                comb_t = bigc.tile([128, _W], _f32)
                nc.vector.tensor_single_scalar(
                    comb_t[:], iota_t[:], idx_t[:, j : j + 1], mybir.AluOpType.is_equal
                )
                disp_t = bigd.tile([128, _W], _u8)
                nc.scalar.activation(
                    disp_t[:], comb_t[:], mybir.ActivationFunctionType.Copy
                )
                r0 = j * 128
                h = _W // 2
                for k in range(2):
                    nc.sync.dma_start(
                        comb_d[r0 : r0 + 128, k * h : (k + 1) * h],
                        comb_t[:, k * h : (k + 1) * h],
                    )
                    nc.scalar.dma_start(
                        disp_d[r0 : r0 + 128, k * h : (k + 1) * h],
                        disp_t[:, k * h : (k + 1) * h],
                    )

    nc.compile()
    return nc


def _get_nc():
    if not _nc_cache:
        _nc_cache.append(_build_nc())
    return _nc_cache[0]


def _get_iota():
    if not _iota_cache:
        _iota_cache.append(
            np.ascontiguousarray(
                np.broadcast_to(np.arange(_W, dtype=np.float32), (128, _W))
            )
        )
    return _iota_cache[0]


def _make_in_maps(domain_ids, mask):
    ids = np.asarray(domain_ids).reshape(_T).astype(np.int64)
    m = np.asarray(mask).reshape(_T).astype(bool)
    mids = np.where(m, 100, ids).astype(np.float32)
    # [p, b] layout: token b*128 + p
    mids_pb = np.ascontiguousarray(mids.reshape(_B, 128).T)
    u_np = np.triu(np.ones((128, 128), dtype=np.float32))
    iota_np = _get_iota()
    # sel8[k, j*128+m] = (k == j): selects row j of the offset matrix and
    # broadcasts it across all 128 output partitions
    sel8_np = np.zeros((_BL, _BL * 128), dtype=np.float32)
    for j in range(_BL):
        sel8_np[j, j * 128 : (j + 1) * 128] = 1.0
    in_maps = []
    for c in range(_NC):
        rot = np.ascontiguousarray(np.roll(mids_pb, -_BL * c, axis=1))
        g = (np.arange(_B) + _BL * c) % _B  # global block of rotated column r
        w_np = (g[:, None] < (_BL * c + np.arange(_BL))[None, :]).astype(np.float32)
        in_maps.append(
            {
                "mids": rot,
                "midst": np.ascontiguousarray(rot.T),
                "w": np.ascontiguousarray(w_np),
                "u": u_np,
                "sel8": sel8_np,
                "iota": iota_np,
            }
        )
    return in_maps


def _assemble(results):
    comb = np.empty((_T, _E, _C), dtype=np.float32)
    disp = np.empty((_T, _E, _C), dtype=bool)
    for c in range(_NC):
        comb[c * _TL : (c + 1) * _TL] = results[c]["comb"].reshape(_TL, _E, _C)
        disp[c * _TL : (c + 1) * _TL] = (
            results[c]["disp"].view(np.bool_).reshape(_TL, _E, _C)
        )
    return comb, disp


def kernel(**inputs):
    nc = _get_nc()
    in_maps = _make_in_maps(inputs["domain_ids"], inputs["mask"])
    r = run_bass_kernel_spmd(nc, in_maps, core_ids=list(range(_NC)))
    comb, disp = _assemble(r.results)
    l_aux = np.zeros((), dtype=np.float32)
    return (l_aux, comb, disp)


def kernel_traced(inputs, trace_cores=None):
    """Dev helper: run with NTFF profiling, return (outputs, BassKernelResults)."""
    nc = _get_nc()
    in_maps = _make_in_maps(inputs["domain_ids"], inputs["mask"])
    r = run_bass_kernel_spmd(
        nc,
        in_maps,
        core_ids=list(range(_NC)),
        trace=True,
        trace_cores=trace_cores or [0],
    )
    comb, disp = _assemble(r.results)
    return (np.zeros((), dtype=np.float32), comb, disp), r


# revision 21
# speedup vs baseline: 1.0481x; 1.0481x over previous
"""MoE domain-gate routing kernel for Trainium2 (8 NeuronCores, token-sharded).

Computes, for T=8192 tokens, E=16 experts, capacity C=512:
  combine[t, e, c] = 1.0 iff token t routed to expert e at capacity slot c
  dispatch = combine != 0
  l_aux = 0.0

Per-core plan (1024 tokens each):
  * Host merges (domain_ids, mask) into a single f32 "mids" stream
    (padded tokens -> sentinel 100), laid out [128 partitions, 64 blocks]
    with the 64 token-blocks ROTATED so each core's own 8 blocks come
    first; a per-core 0/1 prefix matrix W encodes which (rotated) blocks
    precede each local block globally. This keeps the device program
    identical across cores (SPMD) with all per-core variation in data.
  * Device: block-expert counts from a transposed one-hot (blocks on
    partitions) reduced along the free dim; exclusive per-(local block,
    expert) offsets via one matmul with W; within-block inclusive cumsum
    via matmul with upper-triangular ones; offsets broadcast along
    partitions with K=1 matmuls. From the resulting global positions:
    capacity filter, slot index, flat index idx = e*512 + slot (or a
    sentinel if dropped).
  * Output: for each 128-token row block, compare an iota row [0..8191]
    (DMA-loaded constant) against idx per partition -> [128, 8192] f32
    combine tile (+ u8 cast on the scalar engine for dispatch), DMA to
    DRAM on both HWDGE rings. 40MB of output per core; memory-bound.
"""

import numpy as np

import concourse.bacc as bacc
import concourse.mybir as mybir
import concourse.tile as tile
from concourse.bass_utils import run_bass_kernel_spmd

_T, _E, _C = 8192, 16, 512
_NC = 8  # cores
_B = 64  # global 128-token blocks
_BL = 8  # local blocks per core
_TL = _B // _NC * 128  # tokens per core = 1024
_W = _E * _C  # flat output row width = 8192
_SENT = 60000.0  # flat-index sentinel for dropped/padded tokens (fits uint16)

_f32 = mybir.dt.float32
_u8 = mybir.dt.uint8
_u16 = mybir.dt.uint16

_nc_cache = []
_iota_cache = []


def _build_nc():
    nc = bacc.Bacc("TRN2", target_bir_lowering=False, debug=False, num_devices=_NC)

    mids_d = nc.dram_tensor("mids", [128, _B], _f32, kind="ExternalInput")
    midst_d = nc.dram_tensor("midst", [_B, 128], _f32, kind="ExternalInput")
    w_d = nc.dram_tensor("w", [_B, _BL], _f32, kind="ExternalInput")
    u_d = nc.dram_tensor("u", [128, 128], _f32, kind="ExternalInput")
    sel8_d = nc.dram_tensor("sel8", [_BL, _BL * 128], _f32, kind="ExternalInput")
    iota_d = nc.dram_tensor("iota", [128, _W], _u16, kind="ExternalInput")
    comb_d = nc.dram_tensor("comb", [_TL, _W], _f32, kind="ExternalOutput")
    disp_d = nc.dram_tensor("disp", [_TL, _W], _u8, kind="ExternalOutput")

    with tile.TileContext(nc) as tc:
        with (
            tc.tile_pool(name="consts", bufs=1) as cpool,
            tc.tile_pool(name="work", bufs=1) as wpool,
            tc.tile_pool(name="bigc", bufs=6) as bigc,
            tc.tile_pool(name="bigd", bufs=6) as bigd,
            tc.tile_pool(name="psum", bufs=1, space="PSUM") as ppool,
        ):
            # small inputs first on the sync ring (they gate the scan chain);
            # the 2MB iota constant goes on the scalar ring so it never queues
            # ahead of them
            mids_t = cpool.tile([128, _B], _f32)
            midst_t = cpool.tile([_B, 128], _f32)
            w_t = cpool.tile([_B, _BL], _f32)
            u_t = cpool.tile([128, 128], _f32)
            sel8_t = cpool.tile([_BL, _BL * 128], _f32)
            nc.sync.dma_start(mids_t[:], mids_d[:])
            nc.sync.dma_start(midst_t[:], midst_d[:])
            nc.sync.dma_start(w_t[:], w_d[:])
            nc.sync.dma_start(u_t[:], u_d[:])
            nc.sync.dma_start(sel8_t[:], sel8_d[:])
            iota_t = cpool.tile([128, _W], _u16)
            nc.scalar.dma_start(iota_t[:], iota_d[:])

            # transposed one-hot X2[r, e*128+p] = (midsT[r, p] == e) -> block
            # counts bsums[r, e] by reducing over p (innermost)
            x2_t = wpool.tile([_B, _E * 128], _f32)
            x23 = x2_t[:].rearrange("r (e p) -> r e p", p=128)
            for e in range(_E):
                nc.vector.tensor_single_scalar(
                    x23[:, e, :], midst_t[:], float(e), mybir.AluOpType.is_equal
                )
            bsums_t = wpool.tile([_B, _E], _f32)
            nc.vector.reduce_sum(bsums_t[:], x23[:, :, :], axis=mybir.AxisListType.X)

            # local one-hot X[p, j*16+e] = (mids[p, j] == e), j = local block
            x_t = wpool.tile([128, _BL * _E], _f32)
            x3 = x_t[:].rearrange("p (j e) -> p j e", e=_E)
            for e in range(_E):
                nc.vector.tensor_single_scalar(
                    x3[:, :, e], mids_t[:, 0:_BL], float(e), mybir.AluOpType.is_equal
                )

            # exclusive offsets per (local block, expert): W^T @ bsums -> [8, 16]
            o_p = ppool.tile([_BL, _E], _f32)
            nc.tensor.matmul(o_p[:], w_t[:], bsums_t[:], start=True, stop=True)
            o_s = wpool.tile([_BL, _E], _f32)
            nc.vector.tensor_copy(o_s[:], o_p[:])

            # p1 = within-block inclusive cumsum; p2 = offsets broadcast along
            # partitions (K=1 matmuls per local block)
            # p2 = offsets broadcast along partitions; copied to SBUF so the
            # final add reads only one PSUM operand
            p2 = ppool.tile([128, 128], _f32)
            for j in range(_BL):
                nc.tensor.matmul(
                    p2[:, j * _E : (j + 1) * _E],
                    sel8_t[:, j * 128 : (j + 1) * 128],
                    o_s[:],
                    start=True,
                    stop=True,
                )
            o2_t = wpool.tile([128, 128], _f32)
            nc.vector.tensor_copy(o2_t[:], p2[:])
            p1 = ppool.tile([128, 128], _f32)
            nc.tensor.matmul(p1[:], u_t[:], x_t[:], start=True, stop=True)
            pos_t = wpool.tile([128, 128], _f32)
            nc.vector.tensor_add(pos_t[:], p1[:], o2_t[:])

            # capacity filter and slot index
            sel = wpool.tile([128, 128], _f32)
            nc.vector.tensor_single_scalar(
                sel[:], pos_t[:], float(_C), mybir.AluOpType.is_le
            )
            g_t = wpool.tile([128, 128], _f32)
            nc.vector.tensor_mul(g_t[:], sel[:], x_t[:])
            pos1 = wpool.tile([128, 128], _f32)
            nc.vector.tensor_scalar_add(pos1[:], pos_t[:], -1.0)
            locg = wpool.tile([128, 128], _f32)
            nc.vector.tensor_mul(locg[:], pos1[:], g_t[:])
            loc_s = wpool.tile([128, _BL], _f32)
            nc.vector.reduce_sum(
                loc_s[:],
                locg[:].rearrange("p (j e) -> p j e", e=_E),
                axis=mybir.AxisListType.X,
            )
            kept = wpool.tile([128, _BL], _f32)
            nc.vector.reduce_sum(
                kept[:],
                g_t[:].rearrange("p (j e) -> p j e", e=_E),
                axis=mybir.AxisListType.X,
            )
            # idx = (e*512 + slot) if kept else SENT
            t1 = wpool.tile([128, _BL], _f32)
            nc.vector.tensor_single_scalar(
                t1[:], mids_t[:, 0:_BL], float(_C), mybir.AluOpType.mult
            )
            t2 = wpool.tile([128, _BL], _f32)
            nc.vector.tensor_add(t2[:], t1[:], loc_s[:])
            t3 = wpool.tile([128, _BL], _f32)
            nc.vector.tensor_mul(t3[:], t2[:], kept[:])
            t4 = wpool.tile([128, _BL], _f32)
            nc.vector.tensor_scalar(
                t4[:], kept[:], -_SENT, _SENT, mybir.AluOpType.mult, mybir.AluOpType.add
            )
            idx_t = wpool.tile([128, _BL], _f32)
            nc.vector.tensor_add(idx_t[:], t3[:], t4[:])

            # output generation in [128, 4096] half-blocks; comb DMAs on the
            # sync HWDGE ring, disp DMAs on the scalar HWDGE ring
            _H = _W // 2
            for j in range(_BL):
                r0 = j * 128
                for k in range(2):
                    comb_t = bigc.tile([128, _H], _f32)
                    nc.vector.tensor_single_scalar(
                        comb_t[:],
                        iota_t[:, k * _H : (k + 1) * _H],
                        idx_t[:, j : j + 1],
                        mybir.AluOpType.is_equal,
                    )
                    disp_t = bigd.tile([128, _H], _u8)
                    nc.scalar.activation(
                        disp_t[:], comb_t[:], mybir.ActivationFunctionType.Copy
                    )
                    nc.sync.dma_start(
                        comb_d[r0 : r0 + 128, k * _H : (k + 1) * _H], comb_t[:]
                    )
                    nc.scalar.dma_start(
                        disp_d[r0 : r0 + 128, k * _H : (k + 1) * _H], disp_t[:]
                    )

    nc.compile()
    return nc


def _get_nc():
    if not _nc_cache:
        _nc_cache.append(_build_nc())
    return _nc_cache[0]


def _get_iota():
    if not _iota_cache:
        _iota_cache.append(
            np.ascontiguousarray(
                np.broadcast_to(np.arange(_W, dtype=np.uint16), (128, _W))
            )
        )
    return _iota_cache[0]


def _make_in_maps(domain_ids, mask):
    ids = np.asarray(domain_ids).reshape(_T).astype(np.int64)
    m = np.asarray(mask).reshape(_T).astype(bool)
    mids = np.where(m, 100, ids).astype(np.float32)
    # [p, b] layout: token b*128 + p
    mids_pb = np.ascontiguousarray(mids.reshape(_B, 128).T)
    u_np = np.triu(np.ones((128, 128), dtype=np.float32))
    iota_np = _get_iota()
    # sel8[k, j*128+m] = (k == j): selects row j of the offset matrix and
    # broadcasts it across all 128 output partitions
    sel8_np = np.zeros((_BL, _BL * 128), dtype=np.float32)
    for j in range(_BL):
        sel8_np[j, j * 128 : (j + 1) * 128] = 1.0
    in_maps = []
    for c in range(_NC):
        rot = np.ascontiguousarray(np.roll(mids_pb, -_BL * c, axis=1))
        g = (np.arange(_B) + _BL * c) % _B  # global block of rotated column r
        w_np = (g[:, None] < (_BL * c + np.arange(_BL))[None, :]).astype(np.float32)
        in_maps.append(
            {
                "mids": rot,
                "midst": np.ascontiguousarray(rot.T),
                "w": np.ascontiguousarray(w_np),
                "u": u_np,
                "sel8": sel8_np,
                "iota": iota_np,
            }
        )
    return in_maps


def _assemble(results):
    comb = np.empty((_T, _E, _C), dtype=np.float32)
    disp = np.empty((_T, _E, _C), dtype=bool)
    for c in range(_NC):
        comb[c * _TL : (c + 1) * _TL] = results[c]["comb"].reshape(_TL, _E, _C)
        disp[c * _TL : (c + 1) * _TL] = (
            results[c]["disp"].view(np.bool_).reshape(_TL, _E, _C)
        )
    return comb, disp


def kernel(**inputs):
    nc = _get_nc()
    in_maps = _make_in_maps(inputs["domain_ids"], inputs["mask"])
    r = run_bass_kernel_spmd(nc, in_maps, core_ids=list(range(_NC)))
    comb, disp = _assemble(r.results)
    l_aux = np.zeros((), dtype=np.float32)
    return (l_aux, comb, disp)


def kernel_traced(inputs, trace_cores=None):
    """Dev helper: run with NTFF profiling, return (outputs, BassKernelResults)."""
    nc = _get_nc()
    in_maps = _make_in_maps(inputs["domain_ids"], inputs["mask"])
    r = run_bass_kernel_spmd(
        nc,
        in_maps,
        core_ids=list(range(_NC)),
        trace=True,
        trace_cores=trace_cores or [0],
    )
    comb, disp = _assemble(r.results)
    return (np.zeros((), dtype=np.float32), comb, disp), r


# revision 22
# speedup vs baseline: 1.0877x; 1.0378x over previous
"""MoE domain-gate routing kernel for Trainium2 (8 NeuronCores, token-sharded).

Computes, for T=8192 tokens, E=16 experts, capacity C=512:
  combine[t, e, c] = 1.0 iff token t routed to expert e at capacity slot c
  dispatch = combine != 0
  l_aux = 0.0

Per-core plan (1024 tokens each):
  * Host merges (domain_ids, mask) into a single f32 "mids" stream
    (padded tokens -> sentinel 100), laid out [128 partitions, 64 blocks]
    with the 64 token-blocks ROTATED so each core's own 8 blocks come
    first; a per-core 0/1 prefix matrix W encodes which (rotated) blocks
    precede each local block globally. This keeps the device program
    identical across cores (SPMD) with all per-core variation in data.
    All small inputs are packed into one tensor -> one DMA, one sem.
  * Device: per-(halfblock, expert) counts from a transposed one-hot
    reduced along the free dim; exclusive per-(local block, expert)
    offsets via one matmul with W; within-block inclusive cumsum via
    matmul with upper-triangular ones; offsets broadcast along
    partitions with K=8 selector matmuls. From the global positions:
    capacity filter, slot index, flat index idx = e*512 + slot (or a
    sentinel if dropped).
  * Output: for each 128-token half-row-block, compare a uint16 iota row
    [0..8191] (DMA constant on the scalar ring) against idx per
    partition -> [128, 4096] f32 combine tile (+ u8 cast on the scalar
    engine for dispatch), DMA out on both HWDGE rings. 40MB of output
    per core; memory-bound.

Packed "smalls" layout, one [128, 1352] f32 tensor per core:
  cols    0:64   mids      [128, 64]
  cols   64:128  midst2    [128, 64]  (row 2r+h = tokens of block r, half h)
  cols  128:136  w2        [128, 8]   (prefix weights for midst2 rows)
  cols  136:264  u         [128, 128] (upper-triangular ones)
  cols  264:1288 sel8      [8, 1024]  (partition-broadcast selectors)
"""

import numpy as np

import concourse.bacc as bacc
import concourse.mybir as mybir
import concourse.tile as tile
from concourse.bass_utils import run_bass_kernel_spmd

_T, _E, _C = 8192, 16, 512
_NC = 8  # cores
_B = 64  # global 128-token blocks
_BL = 8  # local blocks per core
_TL = _B // _NC * 128  # tokens per core = 1024
_W = _E * _C  # flat output row width = 8192
_SENT = 60000.0  # flat-index sentinel for dropped/padded tokens (fits uint16)
_SM = 1352  # packed smalls width

_f32 = mybir.dt.float32
_u8 = mybir.dt.uint8
_u16 = mybir.dt.uint16

_nc_cache = []
_iota_cache = []


def _build_nc():
    nc = bacc.Bacc("TRN2", target_bir_lowering=False, debug=False, num_devices=_NC)

    sm_d = nc.dram_tensor("smalls", [128, _SM], _f32, kind="ExternalInput")
    iota_d = nc.dram_tensor("iota", [128, _W], _u16, kind="ExternalInput")
    comb_d = nc.dram_tensor("comb", [_TL, _W], _f32, kind="ExternalOutput")
    disp_d = nc.dram_tensor("disp", [_TL, _W], _u8, kind="ExternalOutput")

    with tile.TileContext(nc) as tc:
        with (
            tc.tile_pool(name="consts", bufs=1) as cpool,
            tc.tile_pool(name="work", bufs=1) as wpool,
            tc.tile_pool(name="bigc", bufs=7) as bigc,
            tc.tile_pool(name="bigd", bufs=7) as bigd,
            tc.tile_pool(name="psum", bufs=1, space="PSUM") as ppool,
        ):
            # one DMA for all the small inputs (sync ring); iota constant on
            # the scalar ring so it never queues ahead of the smalls
            sm_t = cpool.tile([128, _SM], _f32)
            nc.sync.dma_start(sm_t[:], sm_d[:])
            iota_t = cpool.tile([128, _W], _u16)
            nc.scalar.dma_start(iota_t[:], iota_d[:])

            mids_t = sm_t[:, 0:64]
            midst_t = sm_t[:, 64:128]
            w_t = sm_t[:, 128:136]
            u_t = sm_t[:, 136:264]
            sel8_t = sm_t[0:_BL, 264:1288]

            # halfblock one-hot X2[q, e*64+p] = (midst2[q, p] == e); counts
            # per (halfblock, expert) by reducing over p (innermost)
            x2_t = wpool.tile([128, _E * 64], _f32)
            x23 = x2_t[:].rearrange("q (e p) -> q e p", p=64)
            for e in range(_E):
                nc.vector.tensor_single_scalar(
                    x23[:, e, :], midst_t, float(e), mybir.AluOpType.is_equal
                )
            bsums_t = wpool.tile([128, _E], _f32)
            nc.vector.reduce_sum(bsums_t[:], x23[:, :, :], axis=mybir.AxisListType.X)

            # local one-hot X[p, j*16+e] = (mids[p, j] == e), j = local block
            x_t = wpool.tile([128, _BL * _E], _f32)
            x3 = x_t[:].rearrange("p (j e) -> p j e", e=_E)
            for e in range(_E):
                nc.vector.tensor_single_scalar(
                    x3[:, :, e], mids_t[:, 0:_BL], float(e), mybir.AluOpType.is_equal
                )

            # exclusive offsets per (local block, expert): W2^T @ bsums [8, 16]
            o_p = ppool.tile([_BL, _E], _f32)
            nc.tensor.matmul(o_p[:], w_t, bsums_t[:], start=True, stop=True)
            o_s = wpool.tile([_BL, _E], _f32)
            nc.vector.tensor_copy(o_s[:], o_p[:])

            # p2 = offsets broadcast along partitions; copied to SBUF so the
            # final add reads only one PSUM operand
            p2 = ppool.tile([128, 128], _f32)
            for j in range(_BL):
                nc.tensor.matmul(
                    p2[:, j * _E : (j + 1) * _E],
                    sel8_t[:, j * 128 : (j + 1) * 128],
                    o_s[:],
                    start=True,
                    stop=True,
                )
            o2_t = wpool.tile([128, 128], _f32)
            nc.vector.tensor_copy(o2_t[:], p2[:])
            p1 = ppool.tile([128, 128], _f32)
            nc.tensor.matmul(p1[:], u_t, x_t[:], start=True, stop=True)
            pos_t = wpool.tile([128, 128], _f32)
            nc.vector.tensor_add(pos_t[:], p1[:], o2_t[:])

            # capacity filter and slot index
            sel = wpool.tile([128, 128], _f32)
            nc.vector.tensor_single_scalar(
                sel[:], pos_t[:], float(_C), mybir.AluOpType.is_le
            )
            g_t = wpool.tile([128, 128], _f32)
            nc.vector.tensor_mul(g_t[:], sel[:], x_t[:])
            pos1 = wpool.tile([128, 128], _f32)
            nc.vector.tensor_scalar_add(pos1[:], pos_t[:], -1.0)
            locg = wpool.tile([128, 128], _f32)
            nc.vector.tensor_mul(locg[:], pos1[:], g_t[:])
            loc_s = wpool.tile([128, _BL], _f32)
            nc.vector.reduce_sum(
                loc_s[:],
                locg[:].rearrange("p (j e) -> p j e", e=_E),
                axis=mybir.AxisListType.X,
            )
            kept = wpool.tile([128, _BL], _f32)
            nc.vector.reduce_sum(
                kept[:],
                g_t[:].rearrange("p (j e) -> p j e", e=_E),
                axis=mybir.AxisListType.X,
            )
            # idx = (e*512 + slot) if kept else SENT
            t1 = wpool.tile([128, _BL], _f32)
            nc.vector.tensor_single_scalar(
                t1[:], mids_t[:, 0:_BL], float(_C), mybir.AluOpType.mult
            )
            t2 = wpool.tile([128, _BL], _f32)
            nc.vector.tensor_add(t2[:], t1[:], loc_s[:])
            t3 = wpool.tile([128, _BL], _f32)
            nc.vector.tensor_mul(t3[:], t2[:], kept[:])
            t4 = wpool.tile([128, _BL], _f32)
            nc.vector.tensor_scalar(
                t4[:], kept[:], -_SENT, _SENT, mybir.AluOpType.mult, mybir.AluOpType.add
            )
            idx_t = wpool.tile([128, _BL], _f32)
            nc.vector.tensor_add(idx_t[:], t3[:], t4[:])

            # output generation in [128, 4096] half-blocks; comb DMAs on the
            # sync HWDGE ring, disp DMAs on the scalar HWDGE ring
            _H = _W // 2
            for j in range(_BL):
                r0 = j * 128
                for k in range(2):
                    comb_t = bigc.tile([128, _H], _f32)
                    nc.vector.tensor_single_scalar(
                        comb_t[:],
                        iota_t[:, k * _H : (k + 1) * _H],
                        idx_t[:, j : j + 1],
                        mybir.AluOpType.is_equal,
                    )
                    disp_t = bigd.tile([128, _H], _u8)
                    nc.scalar.activation(
                        disp_t[:], comb_t[:], mybir.ActivationFunctionType.Copy
                    )
                    nc.sync.dma_start(
                        comb_d[r0 : r0 + 128, k * _H : (k + 1) * _H], comb_t[:]
                    )
                    nc.scalar.dma_start(
                        disp_d[r0 : r0 + 128, k * _H : (k + 1) * _H], disp_t[:]
                    )

    nc.compile()
    return nc


def _get_nc():
    if not _nc_cache:
        _nc_cache.append(_build_nc())
    return _nc_cache[0]


def _get_iota():
    if not _iota_cache:
        _iota_cache.append(
            np.ascontiguousarray(
                np.broadcast_to(np.arange(_W, dtype=np.uint16), (128, _W))
            )
        )
    return _iota_cache[0]


def _make_in_maps(domain_ids, mask):
    ids = np.asarray(domain_ids).reshape(_T).astype(np.int64)
    m = np.asarray(mask).reshape(_T).astype(bool)
    mids = np.where(m, 100, ids).astype(np.float32)
    # [p, b] layout: token b*128 + p
    mids_pb = np.ascontiguousarray(mids.reshape(_B, 128).T)
    u_np = np.triu(np.ones((128, 128), dtype=np.float32))
    iota_np = _get_iota()
    # sel8[k, j*128+m] = (k == j)
    sel8_np = np.zeros((_BL, _BL * 128), dtype=np.float32)
    for j in range(_BL):
        sel8_np[j, j * 128 : (j + 1) * 128] = 1.0
    in_maps = []
    for c in range(_NC):
        rot = np.roll(mids_pb, -_BL * c, axis=1)  # [128, 64]
        # midst2 row 2r+h = tokens [h*64:(h+1)*64] of rotated block r
        rotT = rot.T  # [64 blocks, 128 tokens]
        midst2 = rotT.reshape(_B, 2, 64).reshape(_B * 2, 64)
        g = (np.arange(_B) + _BL * c) % _B  # global block of rotated block r
        w_np = (g[:, None] < (_BL * c + np.arange(_BL))[None, :]).astype(np.float32)
        w2 = np.repeat(w_np, 2, axis=0)  # [128, 8]
        sm = np.zeros((128, _SM), dtype=np.float32)
        sm[:, 0:64] = rot
        sm[:, 64:128] = midst2
        sm[:, 128:136] = w2
        sm[:, 136:264] = u_np
        sm[0:_BL, 264:1288] = sel8_np
        in_maps.append({"smalls": sm, "iota": iota_np})
    return in_maps


def _assemble(results):
    comb = np.empty((_T, _E, _C), dtype=np.float32)
    disp = np.empty((_T, _E, _C), dtype=bool)
    for c in range(_NC):
        comb[c * _TL : (c + 1) * _TL] = results[c]["comb"].reshape(_TL, _E, _C)
        disp[c * _TL : (c + 1) * _TL] = (
            results[c]["disp"].view(np.bool_).reshape(_TL, _E, _C)
        )
    return comb, disp


def kernel(**inputs):
    nc = _get_nc()
    in_maps = _make_in_maps(inputs["domain_ids"], inputs["mask"])
    r = run_bass_kernel_spmd(nc, in_maps, core_ids=list(range(_NC)))
    comb, disp = _assemble(r.results)
    l_aux = np.zeros((), dtype=np.float32)
    return (l_aux, comb, disp)


def kernel_traced(inputs, trace_cores=None):
    """Dev helper: run with NTFF profiling, return (outputs, BassKernelResults)."""
    nc = _get_nc()
    in_maps = _make_in_maps(inputs["domain_ids"], inputs["mask"])
    r = run_bass_kernel_spmd(
        nc,
        in_maps,
        core_ids=list(range(_NC)),
        trace=True,
        trace_cores=trace_cores or [0],
    )
    comb, disp = _assemble(r.results)
    return (np.zeros((), dtype=np.float32), comb, disp), r


# revision 29
# speedup vs baseline: 1.3239x; 1.2171x over previous
"""MoE domain-gate routing kernel for Trainium2 (8 NeuronCores, token-sharded).

Computes, for T=8192 tokens, E=16 experts, capacity C=512:
  combine[t, e, c] = 1.0 iff token t routed to expert e at capacity slot c
  dispatch = combine != 0
  l_aux = 0.0

Per-core plan (1024 tokens each):
  * Host merges (domain_ids, mask) into a single f32 "mids" stream
    (padded tokens -> sentinel 100), laid out [128 partitions, 64 blocks]
    with the 64 token-blocks ROTATED so each core's own 8 blocks come
    first; a per-core 0/1 prefix matrix W encodes which (rotated) blocks
    precede each local block globally. This keeps the device program
    identical across cores (SPMD) with all per-core variation in data.
    All small inputs are packed into one tensor -> one DMA, one sem.
  * Device: per-(halfblock, expert) counts from a transposed one-hot
    reduced along the free dim; exclusive per-(local block, expert)
    offsets via one matmul with W; within-block inclusive cumsum via
    matmul with upper-triangular ones; offsets broadcast along
    partitions with K=8 selector matmuls. From the global positions:
    capacity filter, slot index, flat index idx = e*512 + slot (or a
    sentinel if dropped).
  * Output: for each 128-token half-row-block, compare a uint16 iota row
    [0..8191] (DMA constant on the scalar ring) against idx per
    partition -> [128, 4096] f32 combine tile (+ u8 cast on the scalar
    engine for dispatch), DMA out on both HWDGE rings. 40MB of output
    per core; memory-bound.

Packed "smalls" layout, one [128, 1352] f32 tensor per core:
  cols    0:64   mids      [128, 64]
  cols   64:128  midst2    [128, 64]  (row 2r+h = tokens of block r, half h)
  cols  128:136  w2        [128, 8]   (prefix weights for midst2 rows)
  cols  136:264  u         [128, 128] (upper-triangular ones)
  cols  264:1288 sel8      [8, 1024]  (partition-broadcast selectors)
"""

import numpy as np

import concourse.bacc as bacc
import concourse.mybir as mybir
import concourse.tile as tile
from concourse.bass_utils import run_bass_kernel_spmd

_T, _E, _C = 8192, 16, 512
_NC = 8  # cores
_B = 64  # global 128-token blocks
_BL = 8  # local blocks per core
_TL = _B // _NC * 128  # tokens per core = 1024
_W = _E * _C  # flat output row width = 8192
_SENT = 60000.0  # flat-index sentinel for dropped/padded tokens (fits uint16)
_SM = 1352  # packed smalls width

_f32 = mybir.dt.float32
_u8 = mybir.dt.uint8
_u16 = mybir.dt.uint16

_nc_cache = []
_iota_cache = []


def _build_nc():
    nc = bacc.Bacc("TRN2", target_bir_lowering=False, debug=False, num_devices=_NC)

    sm_d = nc.dram_tensor("smalls", [128, 264], _f32, kind="ExternalInput")
    sel8_d = nc.dram_tensor("sel8", [_BL, _BL * 128], _f32, kind="ExternalInput")
    iota_d = nc.dram_tensor("iota", [128, _W], _u16, kind="ExternalInput")
    comb_d = nc.dram_tensor("comb", [_TL, _W], _f32, kind="ExternalOutput")
    disp_d = nc.dram_tensor("disp", [_TL, _W], _u8, kind="ExternalOutput")

    with tile.TileContext(nc) as tc:
        with (
            tc.tile_pool(name="consts", bufs=1) as cpool,
            tc.tile_pool(name="work", bufs=1) as wpool,
            tc.tile_pool(name="bigc", bufs=7) as bigc,
            tc.tile_pool(name="bigd", bufs=7) as bigd,
            tc.tile_pool(name="psum", bufs=1, space="PSUM") as ppool,
        ):
            # small inputs first (they gate the scan chain); the iota constant
            # on the scalar ring so it never queues ahead of them
            sm_t = cpool.tile([128, 264], _f32)
            nc.sync.dma_start(sm_t[:], sm_d[:])
            sel8_t = cpool.tile([_BL, _BL * 128], _f32)
            nc.sync.dma_start(sel8_t[:], sel8_d[:])
            iota_t = cpool.tile([128, _W], _u16)
            nc.scalar.dma_start(iota_t[:], iota_d[:])

            mids_t = sm_t[:, 0:64]
            midst_t = sm_t[:, 64:128]
            w_t = sm_t[:, 128:136]
            u_t = sm_t[:, 136:264]

            # halfblock one-hot X2[q, e*64+p] = (midst2[q, p] == e); counts
            # per (halfblock, expert) by reducing over p (innermost)
            x2_t = wpool.tile([128, _E * 64], _f32)
            x23 = x2_t[:].rearrange("q (e p) -> q e p", p=64)
            for e in range(_E):
                nc.vector.tensor_single_scalar(
                    x23[:, e, :], midst_t, float(e), mybir.AluOpType.is_equal
                )
            bsums_t = wpool.tile([128, _E], _f32)
            nc.vector.reduce_sum(bsums_t[:], x23[:, :, :], axis=mybir.AxisListType.X)

            # local one-hot X[p, j*16+e] = (mids[p, j] == e), j = local block
            x_t = wpool.tile([128, _BL * _E], _f32)
            x3 = x_t[:].rearrange("p (j e) -> p j e", e=_E)
            for e in range(_E):
                nc.vector.tensor_single_scalar(
                    x3[:, :, e], mids_t[:, 0:_BL], float(e), mybir.AluOpType.is_equal
                )

            # exclusive offsets per (local block, expert): W2^T @ bsums [8, 16]
            o_p = ppool.tile([_BL, _E], _f32)
            nc.tensor.matmul(o_p[:], w_t, bsums_t[:], start=True, stop=True)
            o_s = wpool.tile([_BL, _E], _f32)
            nc.vector.tensor_copy(o_s[:], o_p[:])

            # p2 = offsets broadcast along partitions; copied to SBUF so the
            # final add reads only one PSUM operand
            p2 = ppool.tile([128, 128], _f32)
            for j in range(_BL):
                nc.tensor.matmul(
                    p2[:, j * _E : (j + 1) * _E],
                    sel8_t[:, j * 128 : (j + 1) * 128],
                    o_s[:],
                    start=True,
                    stop=True,
                )  # noqa: E501
            o2_t = wpool.tile([128, 128], _f32)
            nc.vector.tensor_copy(o2_t[:], p2[:])
            p1 = ppool.tile([128, 128], _f32)
            nc.tensor.matmul(p1[:], u_t, x_t[:], start=True, stop=True)
            pos_t = wpool.tile([128, 128], _f32)
            nc.vector.tensor_add(pos_t[:], p1[:], o2_t[:])

            # capacity filter and slot index
            sel = wpool.tile([128, 128], _f32)
            nc.vector.tensor_single_scalar(
                sel[:], pos_t[:], float(_C), mybir.AluOpType.is_le
            )
            g_t = wpool.tile([128, 128], _f32)
            nc.vector.tensor_mul(g_t[:], sel[:], x_t[:])
            pos1 = wpool.tile([128, 128], _f32)
            nc.vector.tensor_scalar_add(pos1[:], pos_t[:], -1.0)
            locg = wpool.tile([128, 128], _f32)
            nc.vector.tensor_mul(locg[:], pos1[:], g_t[:])
            loc_s = wpool.tile([128, _BL], _f32)
            nc.vector.reduce_sum(
                loc_s[:],
                locg[:].rearrange("p (j e) -> p j e", e=_E),
                axis=mybir.AxisListType.X,
            )
            kept = wpool.tile([128, _BL], _f32)
            nc.vector.reduce_sum(
                kept[:],
                g_t[:].rearrange("p (j e) -> p j e", e=_E),
                axis=mybir.AxisListType.X,
            )
            # idx = (e*512 + slot) if kept else SENT
            t1 = wpool.tile([128, _BL], _f32)
            nc.vector.tensor_single_scalar(
                t1[:], mids_t[:, 0:_BL], float(_C), mybir.AluOpType.mult
            )
            t2 = wpool.tile([128, _BL], _f32)
            nc.vector.tensor_add(t2[:], t1[:], loc_s[:])
            t3 = wpool.tile([128, _BL], _f32)
            nc.vector.tensor_mul(t3[:], t2[:], kept[:])
            t4 = wpool.tile([128, _BL], _f32)
            nc.vector.tensor_scalar(
                t4[:], kept[:], -_SENT, _SENT, mybir.AluOpType.mult, mybir.AluOpType.add
            )
            idx_t = wpool.tile([128, _BL], _f32)
            nc.vector.tensor_add(idx_t[:], t3[:], t4[:])

            # output generation in [128, 4096] half-blocks; comb DMAs on the
            # sync HWDGE ring, disp DMAs on the scalar HWDGE ring
            _H = _W // 2
            for j in range(_BL):
                r0 = j * 128
                for k in range(2):
                    comb_t = bigc.tile([128, _H], _f32)
                    nc.vector.tensor_single_scalar(
                        comb_t[:],
                        iota_t[:, k * _H : (k + 1) * _H],
                        idx_t[:, j : j + 1],
                        mybir.AluOpType.is_equal,
                    )
                    disp_t = bigd.tile([128, _H], _u8)
                    nc.scalar.activation(
                        disp_t[:], comb_t[:], mybir.ActivationFunctionType.Copy
                    )
                    nc.sync.dma_start(
                        comb_d[r0 : r0 + 128, k * _H : (k + 1) * _H], comb_t[:]
                    )
                    nc.scalar.dma_start(
                        disp_d[r0 : r0 + 128, k * _H : (k + 1) * _H], disp_t[:]
                    )

    nc.compile()
    return nc


def _get_nc():
    if not _nc_cache:
        _nc_cache.append(_build_nc())
    return _nc_cache[0]


def _get_iota():
    if not _iota_cache:
        _iota_cache.append(
            np.ascontiguousarray(
                np.broadcast_to(np.arange(_W, dtype=np.uint16), (128, _W))
            )
        )
    return _iota_cache[0]


def _make_in_maps(domain_ids, mask):
    ids = np.asarray(domain_ids).reshape(_T).astype(np.int64)
    m = np.asarray(mask).reshape(_T).astype(bool)
    mids = np.where(m, 100, ids).astype(np.float32)
    # [p, b] layout: token b*128 + p
    mids_pb = np.ascontiguousarray(mids.reshape(_B, 128).T)
    u_np = np.triu(np.ones((128, 128), dtype=np.float32))
    # sel8[k, j*128+m] = (k == j)
    sel8_np = np.zeros((_BL, _BL * 128), dtype=np.float32)
    for j in range(_BL):
        sel8_np[j, j * 128 : (j + 1) * 128] = 1.0
    in_maps = []
    for c in range(_NC):
        rot = np.roll(mids_pb, -_BL * c, axis=1)  # [128, 64]
        # midst2 row 2r+h = tokens [h*64:(h+1)*64] of rotated block r
        rotT = rot.T  # [64 blocks, 128 tokens]
        midst2 = rotT.reshape(_B, 2, 64).reshape(_B * 2, 64)
        g = (np.arange(_B) + _BL * c) % _B  # global block of rotated block r
        w_np = (g[:, None] < (_BL * c + np.arange(_BL))[None, :]).astype(np.float32)
        w2 = np.repeat(w_np, 2, axis=0)  # [128, 8]
        sm = np.zeros((128, 264), dtype=np.float32)
        sm[:, 0:64] = rot
        sm[:, 64:128] = midst2
        sm[:, 128:136] = w2
        sm[:, 136:264] = u_np
        in_maps.append({"smalls": sm, "sel8": sel8_np, "iota": _get_iota()})
    return in_maps


def _assemble(results):
    comb = np.empty((_T, _E, _C), dtype=np.float32)
    disp = np.empty((_T, _E, _C), dtype=bool)
    for c in range(_NC):
        comb[c * _TL : (c + 1) * _TL] = results[c]["comb"].reshape(_TL, _E, _C)
        disp[c * _TL : (c + 1) * _TL] = (
            results[c]["disp"].view(np.bool_).reshape(_TL, _E, _C)
        )
    return comb, disp


def kernel(**inputs):
    nc = _get_nc()
    in_maps = _make_in_maps(inputs["domain_ids"], inputs["mask"])
    r = run_bass_kernel_spmd(nc, in_maps, core_ids=list(range(_NC)))
    comb, disp = _assemble(r.results)
    l_aux = np.zeros((), dtype=np.float32)
    return (l_aux, comb, disp)


def kernel_traced(inputs, trace_cores=None):
    """Dev helper: run with NTFF profiling, return (outputs, BassKernelResults)."""
    nc = _get_nc()
    in_maps = _make_in_maps(inputs["domain_ids"], inputs["mask"])
    r = run_bass_kernel_spmd(
        nc,
        in_maps,
        core_ids=list(range(_NC)),
        trace=True,
        trace_cores=trace_cores or [0],
    )
    comb, disp = _assemble(r.results)
    return (np.zeros((), dtype=np.float32), comb, disp), r


# revision 31
# speedup vs baseline: 1.3299x; 1.0045x over previous
"""MoE domain-gate routing kernel for Trainium2 (8 NeuronCores, token-sharded).

Computes, for T=8192 tokens, E=16 experts, capacity C=512:
  combine[t, e, c] = 1.0 iff token t routed to expert e at capacity slot c
  dispatch = combine != 0
  l_aux = 0.0

Per-core plan (1024 tokens each):
  * Host merges (domain_ids, mask) into a single f32 "mids" stream
    (padded tokens -> sentinel 100), laid out [128 partitions, 64 blocks]
    with the 64 token-blocks ROTATED so each core's own 8 blocks come
    first; a per-core 0/1 prefix matrix W encodes which (rotated) blocks
    precede each local block globally. This keeps the device program
    identical across cores (SPMD) with all per-core variation in data.
    All small inputs are packed into one tensor -> one DMA, one sem.
  * Device: per-(halfblock, expert) counts from a transposed one-hot
    reduced along the free dim; exclusive per-(local block, expert)
    offsets via one matmul with W; within-block inclusive cumsum via
    matmul with upper-triangular ones; offsets broadcast along
    partitions with K=8 selector matmuls. From the global positions:
    capacity filter, slot index, flat index idx = e*512 + slot (or a
    sentinel if dropped).
  * Output: for each 128-token half-row-block, compare a uint16 iota row
    [0..8191] (DMA constant on the scalar ring) against idx per
    partition -> [128, 4096] f32 combine tile (+ u8 cast on the scalar
    engine for dispatch), DMA out on both HWDGE rings. 40MB of output
    per core; memory-bound.

Packed "smalls" layout, one [128, 1352] f32 tensor per core:
  cols    0:64   mids      [128, 64]
  cols   64:128  midst2    [128, 64]  (row 2r+h = tokens of block r, half h)
  cols  128:136  w2        [128, 8]   (prefix weights for midst2 rows)
  cols  136:264  u         [128, 128] (upper-triangular ones)
  cols  264:1288 sel8      [8, 1024]  (partition-broadcast selectors)
"""

import numpy as np

import concourse.bacc as bacc
import concourse.mybir as mybir
import concourse.tile as tile
from concourse.bass_utils import run_bass_kernel_spmd

_T, _E, _C = 8192, 16, 512
_NC = 8  # cores
_B = 64  # global 128-token blocks
_BL = 8  # local blocks per core
_TL = _B // _NC * 128  # tokens per core = 1024
_W = _E * _C  # flat output row width = 8192
_SENT = 60000.0  # flat-index sentinel for dropped/padded tokens (fits uint16)
_SM = 1352  # packed smalls width

_f32 = mybir.dt.float32
_u8 = mybir.dt.uint8
_u16 = mybir.dt.uint16

_nc_cache = []
_iota_cache = []


def _build_nc():
    nc = bacc.Bacc("TRN2", target_bir_lowering=False, debug=False, num_devices=_NC)

    sm_d = nc.dram_tensor("smalls", [128, 264], _f32, kind="ExternalInput")
    sel8_d = nc.dram_tensor("sel8", [_BL, _BL * 128], _f32, kind="ExternalInput")
    iota_d = nc.dram_tensor("iota", [128, _W], _u16, kind="ExternalInput")
    comb_d = nc.dram_tensor("comb", [_TL, _W], _f32, kind="ExternalOutput")
    disp_d = nc.dram_tensor("disp", [_TL, _W], _u8, kind="ExternalOutput")

    with tile.TileContext(nc) as tc:
        with (
            tc.tile_pool(name="consts", bufs=1) as cpool,
            tc.tile_pool(name="work", bufs=1) as wpool,
            tc.tile_pool(name="bigc", bufs=7) as bigc,
            tc.tile_pool(name="bigd", bufs=7) as bigd,
            tc.tile_pool(name="psum", bufs=1, space="PSUM") as ppool,
        ):
            # small inputs first (they gate the scan chain); the iota constant
            # on the scalar ring so it never queues ahead of them
            sm_t = cpool.tile([128, 264], _f32)
            nc.sync.dma_start(sm_t[:], sm_d[:])
            sel8_t = cpool.tile([_BL, _BL * 128], _f32)
            nc.sync.dma_start(sel8_t[:], sel8_d[:])
            iota_t = cpool.tile([128, _W], _u16)
            nc.scalar.dma_start(iota_t[:], iota_d[:])

            mids_t = sm_t[:, 0:64]
            midst_t = sm_t[:, 64:128]
            w_t = sm_t[:, 128:136]
            u_t = sm_t[:, 136:264]

            # halfblock one-hot X2[q, e*64+p] = (midst2[q, p] == e); counts
            # per (halfblock, expert) by reducing over p (innermost)
            x2_t = wpool.tile([128, _E * 64], _f32)
            x23 = x2_t[:].rearrange("q (e p) -> q e p", p=64)
            for e in range(_E):
                nc.vector.tensor_single_scalar(
                    x23[:, e, :], midst_t, float(e), mybir.AluOpType.is_equal
                )
            bsums_t = wpool.tile([128, _E], _f32)
            nc.vector.reduce_sum(bsums_t[:], x23[:, :, :], axis=mybir.AxisListType.X)

            # local one-hot X[p, j*16+e] = (mids[p, j] == e), j = local block
            x_t = wpool.tile([128, _BL * _E], _f32)
            x3 = x_t[:].rearrange("p (j e) -> p j e", e=_E)
            for e in range(_E):
                nc.vector.tensor_single_scalar(
                    x3[:, :, e], mids_t[:, 0:_BL], float(e), mybir.AluOpType.is_equal
                )

            # exclusive offsets per (local block, expert): W2^T @ bsums [8, 16]
            o_p = ppool.tile([_BL, _E], _f32)
            nc.tensor.matmul(o_p[:], w_t, bsums_t[:], start=True, stop=True)
            o_s = wpool.tile([_BL, _E], _f32)
            nc.vector.tensor_copy(o_s[:], o_p[:])

            # p2 = offsets broadcast along partitions; copied to SBUF so the
            # final add reads only one PSUM operand
            p2 = ppool.tile([128, 128], _f32)
            for j in range(_BL):
                nc.tensor.matmul(
                    p2[:, j * _E : (j + 1) * _E],
                    sel8_t[:, j * 128 : (j + 1) * 128],
                    o_s[:],
                    start=True,
                    stop=True,
                )  # noqa: E501
            o2_t = wpool.tile([128, 128], _f32)
            nc.vector.tensor_copy(o2_t[:], p2[:])
            p1 = ppool.tile([128, 128], _f32)
            nc.tensor.matmul(p1[:], u_t, x_t[:], start=True, stop=True)
            pos_t = wpool.tile([128, 128], _f32)
            nc.vector.tensor_add(pos_t[:], p1[:], o2_t[:])

            # capacity filter and slot index
            sel = wpool.tile([128, 128], _f32)
            nc.vector.tensor_single_scalar(
                sel[:], pos_t[:], float(_C), mybir.AluOpType.is_le
            )
            g_t = wpool.tile([128, 128], _f32)
            nc.vector.tensor_mul(g_t[:], sel[:], x_t[:])
            pos1 = wpool.tile([128, 128], _f32)
            nc.vector.tensor_scalar_add(pos1[:], pos_t[:], -1.0)
            locg = wpool.tile([128, 128], _f32)
            nc.vector.tensor_mul(locg[:], pos1[:], g_t[:])
            loc_s = wpool.tile([128, _BL], _f32)
            nc.vector.reduce_sum(
                loc_s[:],
                locg[:].rearrange("p (j e) -> p j e", e=_E),
                axis=mybir.AxisListType.X,
            )
            kept = wpool.tile([128, _BL], _f32)
            nc.vector.reduce_sum(
                kept[:],
                g_t[:].rearrange("p (j e) -> p j e", e=_E),
                axis=mybir.AxisListType.X,
            )
            # idx = (e*512 + slot) if kept else SENT
            t1 = wpool.tile([128, _BL], _f32)
            nc.vector.tensor_single_scalar(
                t1[:], mids_t[:, 0:_BL], float(_C), mybir.AluOpType.mult
            )
            t2 = wpool.tile([128, _BL], _f32)
            nc.vector.tensor_add(t2[:], t1[:], loc_s[:])
            t3 = wpool.tile([128, _BL], _f32)
            nc.vector.tensor_mul(t3[:], t2[:], kept[:])
            t4 = wpool.tile([128, _BL], _f32)
            nc.vector.tensor_scalar(
                t4[:], kept[:], -_SENT, _SENT, mybir.AluOpType.mult, mybir.AluOpType.add
            )
            idx_t = wpool.tile([128, _BL], _f32)
            nc.vector.tensor_add(idx_t[:], t3[:], t4[:])

            # output generation in [128, 4096] half-blocks; comb DMAs on the
            # sync HWDGE ring, disp DMAs on the scalar HWDGE ring
            _H = _W // 2
            for j in range(_BL):
                r0 = j * 128
                for k in range(2):
                    comb_t = bigc.tile([128, _H], _f32)
                    nc.vector.tensor_single_scalar(
                        comb_t[:],
                        iota_t[:, k * _H : (k + 1) * _H],
                        idx_t[:, j : j + 1],
                        mybir.AluOpType.is_equal,
                    )
                    disp_t = bigd.tile([128, _H], _u8)
                    nc.scalar.activation(
                        disp_t[:], comb_t[:], mybir.ActivationFunctionType.Copy
                    )
                    nc.sync.dma_start(
                        comb_d[r0 : r0 + 128, k * _H : (k + 1) * _H], comb_t[:]
                    )
                    nc.scalar.dma_start(
                        disp_d[r0 : r0 + 128, k * _H : (k + 1) * _H], disp_t[:]
                    )

    nc.compile()
    return nc


def _get_nc():
    if not _nc_cache:
        _nc_cache.append(_build_nc())
    return _nc_cache[0]


def _get_iota():
    if not _iota_cache:
        _iota_cache.append(
            np.ascontiguousarray(
                np.broadcast_to(np.arange(_W, dtype=np.uint16), (128, _W))
            )
        )
    return _iota_cache[0]


def _make_in_maps(domain_ids, mask):
    ids = np.asarray(domain_ids).reshape(_T).astype(np.int64)
    m = np.asarray(mask).reshape(_T).astype(bool)
    mids = np.where(m, 100, ids).astype(np.float32)
    # [p, b] layout: token b*128 + p
    mids_pb = np.ascontiguousarray(mids.reshape(_B, 128).T)
    u_np = np.triu(np.ones((128, 128), dtype=np.float32))
    # sel8[k, j*128+m] = (k == j)
    sel8_np = np.zeros((_BL, _BL * 128), dtype=np.float32)
    for j in range(_BL):
        sel8_np[j, j * 128 : (j + 1) * 128] = 1.0
    in_maps = []
    for c in range(_NC):
        rot = np.roll(mids_pb, -_BL * c, axis=1)  # [128, 64]
        # midst2 row 2r+h = tokens [h*64:(h+1)*64] of rotated block r
        rotT = rot.T  # [64 blocks, 128 tokens]
        midst2 = rotT.reshape(_B, 2, 64).reshape(_B * 2, 64)
        g = (np.arange(_B) + _BL * c) % _B  # global block of rotated block r
        w_np = (g[:, None] < (_BL * c + np.arange(_BL))[None, :]).astype(np.float32)
        w2 = np.repeat(w_np, 2, axis=0)  # [128, 8]
        sm = np.zeros((128, 264), dtype=np.float32)
        sm[:, 0:64] = rot
        sm[:, 64:128] = midst2
        sm[:, 128:136] = w2
        sm[:, 136:264] = u_np
        in_maps.append({"smalls": sm, "sel8": sel8_np, "iota": _get_iota()})
    return in_maps


def _assemble(results):
    comb = np.empty((_T, _E, _C), dtype=np.float32)
    disp = np.empty((_T, _E, _C), dtype=bool)
    for c in range(_NC):
        comb[c * _TL : (c + 1) * _TL] = results[c]["comb"].reshape(_TL, _E, _C)
        disp[c * _TL : (c + 1) * _TL] = (
            results[c]["disp"].view(np.bool_).reshape(_TL, _E, _C)
        )
    return comb, disp


def kernel(**inputs):
    nc = _get_nc()
    in_maps = _make_in_maps(inputs["domain_ids"], inputs["mask"])
    r = run_bass_kernel_spmd(nc, in_maps, core_ids=list(range(_NC)))
    comb, disp = _assemble(r.results)
    l_aux = np.zeros((), dtype=np.float32)
    return (l_aux, comb, disp)


def kernel_traced(inputs, trace_cores=None):
    """Dev helper: run with NTFF profiling, return (outputs, BassKernelResults)."""
    nc = _get_nc()
    in_maps = _make_in_maps(inputs["domain_ids"], inputs["mask"])
    r = run_bass_kernel_spmd(
        nc,
        in_maps,
        core_ids=list(range(_NC)),
        trace=True,
        trace_cores=trace_cores or [0],
    )
    comb, disp = _assemble(r.results)
    return (np.zeros((), dtype=np.float32), comb, disp), r


# revision 40
# speedup vs baseline: 1.3438x; 1.0104x over previous
"""MoE domain-gate routing kernel for Trainium2 (8 NeuronCores, token-sharded).

Computes, for T=8192 tokens, E=16 experts, capacity C=512:
  combine[t, e, c] = 1.0 iff token t routed to expert e at capacity slot c
  dispatch = combine != 0
  l_aux = 0.0

Per-core plan (1024 tokens each):
  * Host merges (domain_ids, mask) into a single f32 "mids" stream
    (padded tokens -> sentinel 100), laid out [128 partitions, 64 blocks]
    with the 64 token-blocks ROTATED so each core's own 8 blocks come
    first; a per-core 0/1 prefix matrix W encodes which (rotated) blocks
    precede each local block globally. This keeps the device program
    identical across cores (SPMD) with all per-core variation in data.
    All small inputs are packed into one tensor -> one DMA, one sem.
  * Device: per-(halfblock, expert) counts from a transposed one-hot
    reduced along the free dim; exclusive per-(local block, expert)
    offsets via one matmul with W; within-block inclusive cumsum via
    matmul with upper-triangular ones; offsets broadcast along
    partitions with K=8 selector matmuls. From the global positions:
    capacity filter, slot index, flat index idx = e*512 + slot (or a
    sentinel if dropped).
  * Output: for each 128-token half-row-block, compare a uint16 iota row
    [0..8191] (DMA constant on the scalar ring) against idx per
    partition -> [128, 4096] f32 combine tile (+ u8 cast on the scalar
    engine for dispatch), DMA out on both HWDGE rings. 40MB of output
    per core; memory-bound.

Packed "smalls" layout, one [128, 1352] f32 tensor per core:
  cols    0:64   mids      [128, 64]
  cols   64:128  midst2    [128, 64]  (row 2r+h = tokens of block r, half h)
  cols  128:136  w2        [128, 8]   (prefix weights for midst2 rows)
  cols  136:264  u         [128, 128] (upper-triangular ones)
  cols  264:1288 sel8      [8, 1024]  (partition-broadcast selectors)
"""

import numpy as np

import concourse.bacc as bacc
import concourse.mybir as mybir
import concourse.tile as tile
from concourse.bass_utils import run_bass_kernel_spmd
from concourse.tile_rust import add_dep_helper

_T, _E, _C = 8192, 16, 512
_NC = 8  # cores
_B = 64  # global 128-token blocks
_BL = 8  # local blocks per core
_TL = _B // _NC * 128  # tokens per core = 1024
_W = _E * _C  # flat output row width = 8192
_SENT = 60000.0  # flat-index sentinel for dropped/padded tokens (fits uint16)
_SM = 1352  # packed smalls width

_f32 = mybir.dt.float32
_u8 = mybir.dt.uint8
_u16 = mybir.dt.uint16

_nc_cache = []
_iota_cache = []


def _build_nc():
    nc = bacc.Bacc("TRN2", target_bir_lowering=False, debug=False, num_devices=_NC)

    sm_d = nc.dram_tensor("smalls", [128, 264], _f32, kind="ExternalInput")
    sel8_d = nc.dram_tensor("sel8", [_BL, _BL * 128], _f32, kind="ExternalInput")
    iota_d = nc.dram_tensor("iota", [128, _W], _u16, kind="ExternalInput")
    comb_d = nc.dram_tensor("comb", [_TL, _W], _f32, kind="ExternalOutput")
    disp_d = nc.dram_tensor("disp", [_TL, _W], _u8, kind="ExternalOutput")

    with tile.TileContext(nc) as tc:
        with (
            tc.tile_pool(name="consts", bufs=1) as cpool,
            tc.tile_pool(name="work", bufs=1) as wpool,
            tc.tile_pool(name="bigc", bufs=7) as bigc,
            tc.tile_pool(name="bigd", bufs=7) as bigd,
            tc.tile_pool(name="psum", bufs=1, space="PSUM") as ppool,
        ):
            # small inputs first (they gate the scan chain); the iota constant
            # on the scalar ring so it never queues ahead of them
            sm_t = cpool.tile([128, 264], _f32)
            nc.sync.dma_start(sm_t[:], sm_d[:])
            sel8_t = cpool.tile([_BL, _BL * 128], _f32)
            nc.sync.dma_start(sel8_t[:], sel8_d[:])
            iota_t = cpool.tile([128, _W], _u16)
            nc.scalar.dma_start(iota_t[:], iota_d[:])

            mids_t = sm_t[:, 0:64]
            midst_t = sm_t[:, 64:128]
            w_t = sm_t[:, 128:136]
            u_t = sm_t[:, 136:264]

            # per-(halfblock, expert) counts directly via accum_out (the free-
            # dim sum of each one-hot compare) — no separate reduce needed.
            # Emitted first: the offsets chain (bsums -> mm_O -> p2a -> idx0)
            # is the critical path to the first output tile.
            bsums_t = wpool.tile([128, _E], _f32)
            x2scr = wpool.tile([128, 64], _f32)
            for e in range(_E):
                nc.vector.tensor_scalar(
                    x2scr[:],
                    midst_t,
                    float(e),
                    0.0,
                    mybir.AluOpType.is_equal,
                    mybir.AluOpType.add,
                    accum_out=bsums_t[:, e : e + 1],
                )

            # local one-hot X[p, j*16+e] = (mids[p, j] == e), j = local block
            x_t = wpool.tile([128, _BL * _E], _f32)
            x3 = x_t[:].rearrange("p (j e) -> p j e", e=_E)
            for e in range(_E):
                nc.vector.tensor_single_scalar(
                    x3[:, :, e], mids_t[:, 0:_BL], float(e), mybir.AluOpType.is_equal
                )

            # exclusive offsets per (local block, expert): W2^T @ bsums [8, 16]
            o_p = ppool.tile([_BL, _E], _f32)
            nc.tensor.matmul(o_p[:], w_t, bsums_t[:], start=True, stop=True)
            o_s = wpool.tile([_BL, _E], _f32)
            nc.vector.tensor_copy(o_s[:], o_p[:])

            # p1 = within-block inclusive cumsum for all 8 local blocks
            p1 = ppool.tile([128, 128], _f32)
            nc.tensor.matmul(p1[:], u_t, x_t[:], start=True, stop=True)

            # Offsets broadcast along partitions.  Block j=0 gets its own PSUM
            # tile and a thin [128, 16] epilogue so idx[:, 0] is ready ~4us
            # before the remaining blocks' chain -> output DMA starts earlier.
            p2a = ppool.tile([128, _E], _f32)
            nc.tensor.matmul(
                p2a[:], sel8_t[:, 0:128], o_s[:], start=True, stop=True
            )
            p2b = ppool.tile([128, 7 * _E], _f32)
            for j in range(1, _BL):
                nc.tensor.matmul(
                    p2b[:, (j - 1) * _E : j * _E],
                    sel8_t[:, j * 128 : (j + 1) * 128],
                    o_s[:],
                    start=True,
                    stop=True,
                )

            def idx_chain(nb, o2, p1_sl, x_sl, mids_sl, tag):
                # nb = number of blocks handled; all tiles [128, nb*16]
                pos = wpool.tile([128, nb * _E], _f32, tag=f"pos{tag}")
                nc.vector.tensor_add(pos[:], p1_sl, o2[:])
                s = wpool.tile([128, nb * _E], _f32, tag=f"s{tag}")
                nc.vector.tensor_single_scalar(
                    s[:], pos[:], float(_C), mybir.AluOpType.is_le
                )
                g = wpool.tile([128, nb * _E], _f32, tag=f"g{tag}")
                nc.vector.tensor_mul(g[:], s[:], x_sl)
                pg = wpool.tile([128, nb * _E], _f32, tag=f"pg{tag}")
                nc.vector.tensor_mul(pg[:], pos[:], g[:])
                ps = wpool.tile([128, nb], _f32, tag=f"ps{tag}")
                nc.vector.reduce_sum(
                    ps[:],
                    pg[:].rearrange("p (j e) -> p j e", e=_E),
                    axis=mybir.AxisListType.X,
                )
                k = wpool.tile([128, nb], _f32, tag=f"k{tag}")
                nc.vector.reduce_sum(
                    k[:],
                    g[:].rearrange("p (j e) -> p j e", e=_E),
                    axis=mybir.AxisListType.X,
                )
                # slot = sum((pos-1)*g) = ps - k; idx = (e*512 + slot) if kept
                # else SENT
                a = wpool.tile([128, nb], _f32, tag=f"a{tag}")
                nc.vector.tensor_single_scalar(
                    a[:], mids_sl, float(_C), mybir.AluOpType.mult
                )
                b = wpool.tile([128, nb], _f32, tag=f"b{tag}")
                nc.vector.tensor_add(b[:], a[:], ps[:])
                bk = wpool.tile([128, nb], _f32, tag=f"bk{tag}")
                nc.vector.tensor_sub(bk[:], b[:], k[:])  # (a + ps - k)
                c = wpool.tile([128, nb], _f32, tag=f"c{tag}")
                nc.vector.tensor_mul(c[:], bk[:], k[:])
                d = wpool.tile([128, nb], _f32, tag=f"d{tag}")
                nc.vector.tensor_scalar(
                    d[:], k[:], -_SENT, _SENT,
                    mybir.AluOpType.mult, mybir.AluOpType.add,
                )
                idx = wpool.tile([128, nb], _f32, tag=f"idx{tag}")
                nc.vector.tensor_add(idx[:], c[:], d[:])
                return idx

            o2a_t = wpool.tile([128, _E], _f32)
            nc.vector.tensor_copy(o2a_t[:], p2a[:])
            idx0_t = idx_chain(
                1, o2a_t[:], p1[:, 0:_E], x_t[:, 0:_E], mids_t[:, 0:1], "0"
            )

            # output generation in [128, 4096] half-blocks; comb DMAs on the
            # sync HWDGE ring, disp DMAs on the scalar HWDGE ring
            _H = _W // 2
            def emit_output(j, idx_col):
                r0 = j * 128
                first = None
                for k in range(2):
                    comb_t = bigc.tile([128, _H], _f32, tag="comb")
                    ci = nc.vector.tensor_single_scalar(
                        comb_t[:],
                        iota_t[:, k * _H : (k + 1) * _H],
                        idx_col,
                        mybir.AluOpType.is_equal,
                    )
                    if first is None:
                        first = ci
                    disp_t = bigd.tile([128, _H], _u8, tag="disp")
                    nc.scalar.activation(
                        disp_t[:], comb_t[:], mybir.ActivationFunctionType.Copy
                    )
                    nc.sync.dma_start(
                        comb_d[r0 : r0 + 128, k * _H : (k + 1) * _H], comb_t[:]
                    )
                    nc.scalar.dma_start(
                        disp_d[r0 : r0 + 128, k * _H : (k + 1) * _H], disp_t[:]
                    )
                return first

            cmp00 = emit_output(0, idx0_t[:, 0:1])

            # the o2b copy waits on all 7 remaining PE matmuls; order it (and
            # hence the whole in-order DVE tail of the r-chain) after the
            # first j=0 compare so the output stream starts early
            o2b_t = wpool.tile([128, 7 * _E], _f32)
            cb = nc.vector.tensor_copy(o2b_t[:], p2b[:])
            add_dep_helper(
                cb.ins, cmp00.ins, sync=False,
                reason="start output stream before the remaining idx chain",
            )
            idxr_t = idx_chain(
                7, o2b_t[:], p1[:, _E:128], x_t[:, _E:128], mids_t[:, 1:_BL], "r"
            )
            for j in range(1, _BL):
                emit_output(j, idxr_t[:, j - 1 : j])

    nc.compile()
    return nc


def _get_nc():
    if not _nc_cache:
        _nc_cache.append(_build_nc())
    return _nc_cache[0]


def _get_iota():
    if not _iota_cache:
        _iota_cache.append(
            np.ascontiguousarray(
                np.broadcast_to(np.arange(_W, dtype=np.uint16), (128, _W))
            )
        )
    return _iota_cache[0]


def _make_in_maps(domain_ids, mask):
    ids = np.asarray(domain_ids).reshape(_T).astype(np.int64)
    m = np.asarray(mask).reshape(_T).astype(bool)
    mids = np.where(m, 100, ids).astype(np.float32)
    # [p, b] layout: token b*128 + p
    mids_pb = np.ascontiguousarray(mids.reshape(_B, 128).T)
    u_np = np.triu(np.ones((128, 128), dtype=np.float32))
    # sel8[k, j*128+m] = (k == j)
    sel8_np = np.zeros((_BL, _BL * 128), dtype=np.float32)
    for j in range(_BL):
        sel8_np[j, j * 128 : (j + 1) * 128] = 1.0
    in_maps = []
    for c in range(_NC):
        rot = np.roll(mids_pb, -_BL * c, axis=1)  # [128, 64]
        # midst2 row 2r+h = tokens [h*64:(h+1)*64] of rotated block r
        rotT = rot.T  # [64 blocks, 128 tokens]
        midst2 = rotT.reshape(_B, 2, 64).reshape(_B * 2, 64)
        g = (np.arange(_B) + _BL * c) % _B  # global block of rotated block r
        w_np = (g[:, None] < (_BL * c + np.arange(_BL))[None, :]).astype(np.float32)
        w2 = np.repeat(w_np, 2, axis=0)  # [128, 8]
        sm = np.zeros((128, 264), dtype=np.float32)
        sm[:, 0:64] = rot
        sm[:, 64:128] = midst2
        sm[:, 128:136] = w2
        sm[:, 136:264] = u_np
        in_maps.append({"smalls": sm, "sel8": sel8_np, "iota": _get_iota()})
    return in_maps


def _assemble(results):
    comb = np.empty((_T, _E, _C), dtype=np.float32)
    disp = np.empty((_T, _E, _C), dtype=bool)
    for c in range(_NC):
        comb[c * _TL : (c + 1) * _TL] = results[c]["comb"].reshape(_TL, _E, _C)
        disp[c * _TL : (c + 1) * _TL] = (
            results[c]["disp"].view(np.bool_).reshape(_TL, _E, _C)
        )
    return comb, disp


def kernel(**inputs):
    nc = _get_nc()
    in_maps = _make_in_maps(inputs["domain_ids"], inputs["mask"])
    r = run_bass_kernel_spmd(nc, in_maps, core_ids=list(range(_NC)))
    comb, disp = _assemble(r.results)
    l_aux = np.zeros((), dtype=np.float32)
    return (l_aux, comb, disp)


def kernel_traced(inputs, trace_cores=None):
    """Dev helper: run with NTFF profiling, return (outputs, BassKernelResults)."""
    nc = _get_nc()
    in_maps = _make_in_maps(inputs["domain_ids"], inputs["mask"])
    r = run_bass_kernel_spmd(
        nc,
        in_maps,
        core_ids=list(range(_NC)),
        trace=True,
        trace_cores=trace_cores or [0],
    )
    comb, disp = _assemble(r.results)
    return (np.zeros((), dtype=np.float32), comb, disp), r


# revision 46
# speedup vs baseline: 1.3535x; 1.0072x over previous
"""MoE domain-gate routing kernel for Trainium2 (8 NeuronCores, token-sharded).

Computes, for T=8192 tokens, E=16 experts, capacity C=512:
  combine[t, e, c] = 1.0 iff token t routed to expert e at capacity slot c
  dispatch = combine != 0
  l_aux = 0.0

Per-core plan (1024 tokens each):
  * Host merges (domain_ids, mask) into a single f32 "mids" stream
    (padded tokens -> sentinel 100), laid out [128 partitions, 64 blocks]
    with the 64 token-blocks ROTATED so each core's own 8 blocks come
    first; a per-core 0/1 prefix matrix W encodes which (rotated) blocks
    precede each local block globally. This keeps the device program
    identical across cores (SPMD) with all per-core variation in data.
    All small inputs are packed into one tensor -> one DMA, one sem.
  * Device: per-(halfblock, expert) counts from a transposed one-hot
    reduced along the free dim; exclusive per-(local block, expert)
    offsets via one matmul with W; within-block inclusive cumsum via
    matmul with upper-triangular ones; offsets broadcast along
    partitions with K=8 selector matmuls. From the global positions:
    capacity filter, slot index, flat index idx = e*512 + slot (or a
    sentinel if dropped).
  * Output: for each 128-token half-row-block, compare a uint16 iota row
    [0..8191] (DMA constant on the scalar ring) against idx per
    partition -> [128, 4096] f32 combine tile (+ u8 cast on the scalar
    engine for dispatch), DMA out on both HWDGE rings. 40MB of output
    per core; memory-bound.

Packed "smalls" layout, one [128, 1352] f32 tensor per core:
  cols    0:64   mids      [128, 64]
  cols   64:128  midst2    [128, 64]  (row 2r+h = tokens of block r, half h)
  cols  128:136  w2        [128, 8]   (prefix weights for midst2 rows)
  cols  136:264  u         [128, 128] (upper-triangular ones)
  cols  264:1288 sel8      [8, 1024]  (partition-broadcast selectors)
"""

import numpy as np

import concourse.bacc as bacc
import concourse.mybir as mybir
import concourse.tile as tile
from concourse.bass_utils import run_bass_kernel_spmd
from concourse.tile_rust import add_dep_helper

_T, _E, _C = 8192, 16, 512
_NC = 8  # cores
_B = 64  # global 128-token blocks
_BL = 8  # local blocks per core
_TL = _B // _NC * 128  # tokens per core = 1024
_W = _E * _C  # flat output row width = 8192
_SENT = 60000.0  # flat-index sentinel for dropped/padded tokens (fits uint16)
_SM = 1352  # packed smalls width

_f32 = mybir.dt.float32
_u8 = mybir.dt.uint8
_u16 = mybir.dt.uint16

_nc_cache = []
_iota_cache = []


def _build_nc():
    nc = bacc.Bacc("TRN2", target_bir_lowering=False, debug=False, num_devices=_NC)

    sm_d = nc.dram_tensor("smalls", [128, 264], _f32, kind="ExternalInput")
    sel8_d = nc.dram_tensor("sel8", [_BL, _BL * 128], _f32, kind="ExternalInput")
    iota_d = nc.dram_tensor("iota", [128, _W // 2], _u16, kind="ExternalInput")
    comb_d = nc.dram_tensor("comb", [_TL, _W], _f32, kind="ExternalOutput")
    disp_d = nc.dram_tensor("disp", [_TL, _W], _u8, kind="ExternalOutput")

    with tile.TileContext(nc) as tc:
        with (
            tc.tile_pool(name="consts", bufs=1) as cpool,
            tc.tile_pool(name="work", bufs=1) as wpool,
            tc.tile_pool(name="bigc", bufs=7) as bigc,
            tc.tile_pool(name="bigd", bufs=7) as bigd,
            tc.tile_pool(name="psum", bufs=1, space="PSUM") as ppool,
        ):
            # small inputs first (they gate the scan chain); the iota constant
            # on the scalar ring so it never queues ahead of them
            sm_t = cpool.tile([128, 264], _f32)
            nc.sync.dma_start(sm_t[:], sm_d[:])
            sel8_t = cpool.tile([_BL, _BL * 128], _f32)
            nc.sync.dma_start(sel8_t[:], sel8_d[:])
            iota_t = cpool.tile([128, _W // 2], _u16)
            nc.scalar.dma_start(iota_t[:], iota_d[:])

            mids_t = sm_t[:, 0:64]
            midst_t = sm_t[:, 64:128]
            w_t = sm_t[:, 128:136]
            u_t = sm_t[:, 136:264]

            # per-(halfblock, expert) counts directly via accum_out (the free-
            # dim sum of each one-hot compare) — no separate reduce needed.
            # Emitted first: the offsets chain (bsums -> mm_O -> p2a -> idx0)
            # is the critical path to the first output tile.
            bsums_t = wpool.tile([128, _E], _f32)
            x2scr = wpool.tile([128, 64], _f32)
            for e in range(_E):
                nc.vector.tensor_scalar(
                    x2scr[:],
                    midst_t,
                    float(e),
                    0.0,
                    mybir.AluOpType.is_equal,
                    mybir.AluOpType.add,
                    accum_out=bsums_t[:, e : e + 1],
                )

            # local one-hot X[p, j*16+e] = (mids[p, j] == e), j = local block
            x_t = wpool.tile([128, _BL * _E], _f32)
            x3 = x_t[:].rearrange("p (j e) -> p j e", e=_E)
            for e in range(_E):
                nc.vector.tensor_single_scalar(
                    x3[:, :, e], mids_t[:, 0:_BL], float(e), mybir.AluOpType.is_equal
                )

            # exclusive offsets per (local block, expert): W2^T @ bsums [8, 16]
            o_p = ppool.tile([_BL, _E], _f32)
            nc.tensor.matmul(o_p[:], w_t, bsums_t[:], start=True, stop=True)
            o_s = wpool.tile([_BL, _E], _f32)
            nc.vector.tensor_copy(o_s[:], o_p[:])

            # p1 = within-block inclusive cumsum for all 8 local blocks
            p1 = ppool.tile([128, 128], _f32)
            nc.tensor.matmul(p1[:], u_t, x_t[:], start=True, stop=True)

            # Offsets broadcast along partitions.  Block j=0 gets its own PSUM
            # tile and a thin [128, 16] epilogue so idx[:, 0] is ready ~4us
            # before the remaining blocks' chain -> output DMA starts earlier.
            p2a = ppool.tile([128, _E], _f32)
            nc.tensor.matmul(
                p2a[:], sel8_t[:, 0:128], o_s[:], start=True, stop=True
            )
            p2b = ppool.tile([128, 7 * _E], _f32)
            for j in range(1, _BL):
                nc.tensor.matmul(
                    p2b[:, (j - 1) * _E : j * _E],
                    sel8_t[:, j * 128 : (j + 1) * 128],
                    o_s[:],
                    start=True,
                    stop=True,
                )

            def idx_chain(nb, o2, p1_sl, x_sl, mids_sl, tag):
                # nb = number of blocks handled; all tiles [128, nb*16]
                pos = wpool.tile([128, nb * _E], _f32, tag=f"pos{tag}")
                nc.vector.tensor_add(pos[:], p1_sl, o2[:])
                s = wpool.tile([128, nb * _E], _f32, tag=f"s{tag}")
                nc.vector.tensor_single_scalar(
                    s[:], pos[:], float(_C), mybir.AluOpType.is_le
                )
                g = wpool.tile([128, nb * _E], _f32, tag=f"g{tag}")
                nc.vector.tensor_mul(g[:], s[:], x_sl)
                pg = wpool.tile([128, nb * _E], _f32, tag=f"pg{tag}")
                nc.vector.tensor_mul(pg[:], pos[:], g[:])
                ps = wpool.tile([128, nb], _f32, tag=f"ps{tag}")
                nc.vector.reduce_sum(
                    ps[:],
                    pg[:].rearrange("p (j e) -> p j e", e=_E),
                    axis=mybir.AxisListType.X,
                )
                k = wpool.tile([128, nb], _f32, tag=f"k{tag}")
                nc.vector.reduce_sum(
                    k[:],
                    g[:].rearrange("p (j e) -> p j e", e=_E),
                    axis=mybir.AxisListType.X,
                )
                # slot = sum((pos-1)*g) = ps - k; idx = (e*512 + slot) if kept
                # else SENT
                a = wpool.tile([128, nb], _f32, tag=f"a{tag}")
                nc.vector.tensor_single_scalar(
                    a[:], mids_sl, float(_C), mybir.AluOpType.mult
                )
                b = wpool.tile([128, nb], _f32, tag=f"b{tag}")
                nc.vector.tensor_add(b[:], a[:], ps[:])
                bk = wpool.tile([128, nb], _f32, tag=f"bk{tag}")
                nc.vector.tensor_sub(bk[:], b[:], k[:])  # (a + ps - k)
                c = wpool.tile([128, nb], _f32, tag=f"c{tag}")
                nc.vector.tensor_mul(c[:], bk[:], k[:])
                d = wpool.tile([128, nb], _f32, tag=f"d{tag}")
                nc.vector.tensor_scalar(
                    d[:], k[:], -_SENT, _SENT,
                    mybir.AluOpType.mult, mybir.AluOpType.add,
                )
                idx = wpool.tile([128, nb], _f32, tag=f"idx{tag}")
                nc.vector.tensor_add(idx[:], c[:], d[:])
                return idx

            o2a_t = wpool.tile([128, _E], _f32)
            nc.vector.tensor_copy(o2a_t[:], p2a[:])
            idx0_t = idx_chain(
                1, o2a_t[:], p1[:, 0:_E], x_t[:, 0:_E], mids_t[:, 0:1], "0"
            )

            # output generation in [128, 4096] half-blocks; comb DMAs on the
            # sync HWDGE ring, disp DMAs on the scalar HWDGE ring
            _H = _W // 2
            def emit_output(j, idx_col, idxm_col):
                # column f in [H, 2H) matches idx iff f-H == idx-H, so both
                # half-tiles compare against the same [128, H] iota — with the
                # scalar shifted by -H for the upper half (the sentinel and
                # out-of-range values still match nothing)
                r0 = j * 128
                first = None
                for k in range(2):
                    comb_t = bigc.tile([128, _H], _f32, tag="comb")
                    ci = nc.vector.tensor_single_scalar(
                        comb_t[:],
                        iota_t[:],
                        idx_col if k == 0 else idxm_col,
                        mybir.AluOpType.is_equal,
                    )
                    if first is None:
                        first = ci
                    disp_t = bigd.tile([128, _H], _u8, tag="disp")
                    nc.scalar.activation(
                        disp_t[:], comb_t[:], mybir.ActivationFunctionType.Copy
                    )
                    nc.sync.dma_start(
                        comb_d[r0 : r0 + 128, k * _H : (k + 1) * _H], comb_t[:]
                    )
                    nc.scalar.dma_start(
                        disp_d[r0 : r0 + 128, k * _H : (k + 1) * _H], disp_t[:]
                    )
                return first

            idxm0_t = wpool.tile([128, 1], _f32)
            nc.vector.tensor_scalar_add(idxm0_t[:], idx0_t[:], float(-_W // 2))
            cmp00 = emit_output(0, idx0_t[:, 0:1], idxm0_t[:, 0:1])

            # the o2b copy waits on all 7 remaining PE matmuls; order it (and
            # hence the whole in-order DVE tail of the r-chain) after the
            # first j=0 compare so the output stream starts early
            o2b_t = wpool.tile([128, 7 * _E], _f32)
            cb = nc.vector.tensor_copy(o2b_t[:], p2b[:])
            add_dep_helper(
                cb.ins, cmp00.ins, sync=False,
                reason="start output stream before the remaining idx chain",
            )
            idxr_t = idx_chain(
                7, o2b_t[:], p1[:, _E:128], x_t[:, _E:128], mids_t[:, 1:_BL], "r"
            )
            idxmr_t = wpool.tile([128, 7], _f32)
            nc.vector.tensor_scalar_add(idxmr_t[:], idxr_t[:], float(-_W // 2))
            for j in range(1, _BL):
                emit_output(j, idxr_t[:, j - 1 : j], idxmr_t[:, j - 1 : j])

    nc.compile()
    return nc


def _get_nc():
    if not _nc_cache:
        _nc_cache.append(_build_nc())
    return _nc_cache[0]


def _get_iota():
    if not _iota_cache:
        _iota_cache.append(
            np.ascontiguousarray(
                np.broadcast_to(np.arange(_W // 2, dtype=np.uint16), (128, _W // 2))
            )
        )
    return _iota_cache[0]


def _make_in_maps(domain_ids, mask):
    ids = np.asarray(domain_ids).reshape(_T).astype(np.int64)
    m = np.asarray(mask).reshape(_T).astype(bool)
    mids = np.where(m, 100, ids).astype(np.float32)
    # [p, b] layout: token b*128 + p
    mids_pb = np.ascontiguousarray(mids.reshape(_B, 128).T)
    u_np = np.triu(np.ones((128, 128), dtype=np.float32))
    # sel8[k, j*128+m] = (k == j)
    sel8_np = np.zeros((_BL, _BL * 128), dtype=np.float32)
    for j in range(_BL):
        sel8_np[j, j * 128 : (j + 1) * 128] = 1.0
    in_maps = []
    for c in range(_NC):
        rot = np.roll(mids_pb, -_BL * c, axis=1)  # [128, 64]
        # midst2 row 2r+h = tokens [h*64:(h+1)*64] of rotated block r
        rotT = rot.T  # [64 blocks, 128 tokens]
        midst2 = rotT.reshape(_B, 2, 64).reshape(_B * 2, 64)
        g = (np.arange(_B) + _BL * c) % _B  # global block of rotated block r
        w_np = (g[:, None] < (_BL * c + np.arange(_BL))[None, :]).astype(np.float32)
        w2 = np.repeat(w_np, 2, axis=0)  # [128, 8]
        sm = np.zeros((128, 264), dtype=np.float32)
        sm[:, 0:64] = rot
        sm[:, 64:128] = midst2
        sm[:, 128:136] = w2
        sm[:, 136:264] = u_np
        in_maps.append({"smalls": sm, "sel8": sel8_np, "iota": _get_iota()})
    return in_maps


def _assemble(results):
    comb = np.empty((_T, _E, _C), dtype=np.float32)
    disp = np.empty((_T, _E, _C), dtype=bool)
    for c in range(_NC):
        comb[c * _TL : (c + 1) * _TL] = results[c]["comb"].reshape(_TL, _E, _C)
        disp[c * _TL : (c + 1) * _TL] = (
            results[c]["disp"].view(np.bool_).reshape(_TL, _E, _C)
        )
    return comb, disp


def kernel(**inputs):
    nc = _get_nc()
    in_maps = _make_in_maps(inputs["domain_ids"], inputs["mask"])
    r = run_bass_kernel_spmd(nc, in_maps, core_ids=list(range(_NC)))
    comb, disp = _assemble(r.results)
    l_aux = np.zeros((), dtype=np.float32)
    return (l_aux, comb, disp)


def kernel_traced(inputs, trace_cores=None):
    """Dev helper: run with NTFF profiling, return (outputs, BassKernelResults)."""
    nc = _get_nc()
    in_maps = _make_in_maps(inputs["domain_ids"], inputs["mask"])
    r = run_bass_kernel_spmd(
        nc,
        in_maps,
        core_ids=list(range(_NC)),
        trace=True,
        trace_cores=trace_cores or [0],
    )
    comb, disp = _assemble(r.results)
    return (np.zeros((), dtype=np.float32), comb, disp), r
